# revision 60
# baseline (speedup 1.0000x reference)
"""Multi-headed attention (B=8, S=1024, D=1024, H=16) on 8 TRN2 NeuronCores.

Strategy: pure data parallelism over the batch — core b computes batch element b
end-to-end (no collectives). All matmuls in bf16 (fp32 PSUM accumulation).

Per-core dataflow (everything "T" is feature-major [D, S]):
  inputs (host-pretransposed, bf16): qT, keyT, valT, wkT, wvT, woT
  1. K_T[d_out, s]  = matmul(lhsT=wkT, rhs=keyT) + bk      (bias per-partition)
  2. V[s, d_out]    = matmul(lhsT=valT, rhs=wvT) + bv      -> packed [s, h, 65]
                      with a ones column per head (gives softmax denominators
                      for free inside the p@v matmul)
  3. per head h: scoresT[k, q] = matmul(lhsT=K_T_h[64,128], rhs=qT_h[64,512])
                 pT = exp(scoresT / 8)  (ACT; max-subtraction skipped — scores
                 are provably small for this problem)
  4. xT_h[65, q] accum = matmul(lhsT=[V_h | 1][128,65], rhs=pT[128,512]);
     row 64 = softmax denominator; normalize rows 0..63 by its reciprocal
     (partition-broadcast via DMA)
  5. O[s, d_out] = matmul(lhsT=xT, rhs=woT) + bo -> DMA out (f32)
"""

import numpy as np
import ml_dtypes

import concourse.bass as bass
import concourse.bacc as bacc
import concourse.mybir as mybir
import concourse.tile as tile
from contextlib import ExitStack

B, S, D, H = 8, 1024, 1024, 16
P = 128
DK = D // H          # 64
NCH = D // P         # 8
QC = 512             # free-dim chunk (one PSUM bank)
NQC = S // QC        # 2
SCALE = 1.0 / float(np.sqrt(DK))  # 0.125
N_CORES = 8

BF16 = mybir.dt.bfloat16
F32 = mybir.dt.float32
ADD = mybir.AluOpType.add
MULT = mybir.AluOpType.mult
EXP = mybir.ActivationFunctionType.Exp

_CACHE = {}


def _build_nc():
    nc = bacc.Bacc(None)

    qT_d = nc.dram_tensor("qT", [NCH, P, S], BF16, kind="ExternalInput")
    keyT_d = nc.dram_tensor("keyT", [NCH, P, S], BF16, kind="ExternalInput")
    valT_d = nc.dram_tensor("valT", [NCH, P, S], BF16, kind="ExternalInput")
    wkT_d = nc.dram_tensor("wkT", [NCH, P, D], BF16, kind="ExternalInput")
    wvT_d = nc.dram_tensor("wvT", [NCH, P, D], BF16, kind="ExternalInput")
    woT_d = nc.dram_tensor("woT", [NCH, P, D], BF16, kind="ExternalInput")
    bk_d = nc.dram_tensor("bk", [D], F32, kind="ExternalInput")
    bv_d = nc.dram_tensor("bv", [D], F32, kind="ExternalInput")
    bo_d = nc.dram_tensor("bo", [D], F32, kind="ExternalInput")
    out_d = nc.dram_tensor("out", [S, D], F32, kind="ExternalOutput")

    with tile.TileContext(nc) as tc:
        with ExitStack() as ctx:
            const = ctx.enter_context(tc.tile_pool(name="const", bufs=1))
            big = ctx.enter_context(tc.tile_pool(name="big", bufs=1))
            wpool = ctx.enter_context(tc.tile_pool(name="wpool", bufs=1))
            ppool = ctx.enter_context(tc.tile_pool(name="ppool", bufs=2))
            opool = ctx.enter_context(tc.tile_pool(name="opool", bufs=4))
            rpool = ctx.enter_context(tc.tile_pool(name="rpool", bufs=2))
            xpool = ctx.enter_context(tc.tile_pool(name="xpool", bufs=10))
            proj_ps = ctx.enter_context(
                tc.tile_pool(name="proj_ps", bufs=2, space="PSUM")
            )
            sc_ps = ctx.enter_context(tc.tile_pool(name="sc_ps", bufs=2, space="PSUM"))
            xt_ps = ctx.enter_context(tc.tile_pool(name="xt_ps", bufs=2, space="PSUM"))

            # --- SBUF resident tensors ---
            qT = big.tile([P, NCH, S], BF16, tag="qT")
            keyT = big.tile([P, NCH, S], BF16, tag="share1")  # reused later by xT
            # valT in two s-halves; the same 2-slot/16KB tag later hosts the
            # paired-head pT tiles (valT is dead after the V projection)
            valT_a = big.tile([P, NCH, QC], BF16, tag="ptv", bufs=2)
            valT_b = big.tile([P, NCH, QC], BF16, tag="ptv", bufs=2)
            kT = big.tile([P, NCH, S], BF16, tag="kT")
            vpad = big.tile([P, NCH, H, DK + 1], BF16, tag="vpad")
            wk = wpool.tile([P, NCH, D], BF16, tag="wk")
            wv = wpool.tile([P, NCH, D], BF16, tag="wv")
            wo = wpool.tile([P, NCH, D], BF16, tag="wo")
            bk_sb = const.tile([P, NCH], F32, tag="bk")
            bv_b = const.tile([P, D], F32, tag="bv")
            bo_b = const.tile([P, D], F32, tag="bo")

            # --- input DMAs (V-proj deps first: it must fully precede p@v) ---
            nc.sync.dma_start(
                out=valT_a[:], in_=valT_d[:, :, 0:QC].rearrange("c p f -> p c f")
            )
            nc.sync.dma_start(
                out=wv[:, :, 0:QC], in_=wvT_d[:, :, 0:QC].rearrange("c p f -> p c f")
            )
            nc.sync.dma_start(
                out=valT_b[:], in_=valT_d[:, :, QC:S].rearrange("c p f -> p c f")
            )
            nc.sync.dma_start(
                out=wv[:, :, QC:D], in_=wvT_d[:, :, QC:D].rearrange("c p f -> p c f")
            )
            nc.sync.dma_start(out=bv_b[:], in_=bv_d[:][None, :].to_broadcast((P, D)))
            nc.sync.dma_start(out=keyT[:], in_=keyT_d[:].rearrange("c p f -> p c f"))
            nc.sync.dma_start(out=wk[:], in_=wkT_d[:].rearrange("c p f -> p c f"))
            nc.sync.dma_start(out=bk_sb[:], in_=bk_d[:].rearrange("(c p) -> p c", p=P))
            nc.sync.dma_start(out=qT[:], in_=qT_d[:].rearrange("c p f -> p c f"))
            nc.sync.dma_start(out=wo[:], in_=woT_d[:].rearrange("c p f -> p c f"))
            nc.sync.dma_start(out=bo_b[:], in_=bo_d[:][None, :].to_broadcast((P, D)))

            # --- 1. K_T = Wk @ key.T + bk  (feature-major) ---
            # m-tile 0 runs in the prologue; tiles 1..7 are interleaved into
            # attention chunk 0 as PE filler (head pair m needs only tile m).
            ktp_ps = {}

            def emit_ktproj(m, half=None):
                if half in (0, None):
                    ktp_ps[m] = [
                        proj_ps.tile([P, QC], F32, tag="pp", name=f"kp{m}_{j}")
                        for j in range(NQC)
                    ]
                ps = ktp_ps[m]
                cs = range(NCH) if half is None else range(half * 4, half * 4 + 4)
                for c in cs:  # d_in chunk (contraction)
                    st = wk[:, c, m * P : (m + 1) * P]
                    for j in range(NQC):
                        nc.tensor.matmul(
                            ps[j][:],
                            st,
                            keyT[:, c, j * QC : (j + 1) * QC],
                            start=(c == 0),
                            stop=(c == NCH - 1),
                        )
                if half in (1, None):
                    for j in range(NQC):
                        nc.vector.tensor_scalar_add(
                            kT[:, m, j * QC : (j + 1) * QC],
                            ps[j][:],
                            bk_sb[:, m : m + 1],
                        )

            # --- 2. V = value @ Wv.T + bv (token-major, head-padded w/ ones) ---
            # tiles t0-3 run in the prologue; t4-7 hide inside attention
            # steps 0-1 as PE filler (p@v starts at step 1 and consumes
            # k-chunks 4-7 only after those tiles' evictions are emitted)
            vp_ps = {}

            def emit_vproj(t, half):
                if half == 0:
                    vp_ps[t] = [
                        proj_ps.tile([P, QC], F32, tag="pp", name=f"vp{t}_{j}")
                        for j in range(NQC)
                    ]
                ps = vp_ps[t]
                vhalf = valT_a if t < 4 else valT_b
                j = half
                for c in range(NCH):
                    nc.tensor.matmul(
                        ps[j][:],
                        vhalf[:, c, (t % 4) * P : (t % 4 + 1) * P],
                        wv[:, c, j * QC : (j + 1) * QC],
                        start=(c == 0),
                        stop=(c == NCH - 1),
                    )
                if half == 0:
                    return
                hpc = QC // DK  # heads per psum chunk
                for j in range(NQC):
                    nc.vector.tensor_tensor(
                        vpad[:, t, j * hpc : (j + 1) * hpc, 0:DK],
                        ps[j][:].rearrange("p (h d) -> p h d", d=DK),
                        bv_b[:, j * QC : (j + 1) * QC].rearrange(
                            "p (h d) -> p h d", d=DK
                        ),
                        op=ADD,
                    )
                nc.vector.memset(vpad[:, t, :, DK : DK + 1], 1.0)

            for t in range(4):
                emit_vproj(t, 0)
                emit_vproj(t, 1)

            emit_ktproj(0)  # head pair 0's K_T; tiles 1..7 interleave below

            # --- 3+4. per-head attention, q-chunk-outer, head-pipelined ---
            # PE is in-order: emit scores(h) before p@v(h-1) so the PE has
            # work while ACT chews through exp(h-1). O-projection s-tiles
            # for q-chunk j unlock once all heads finished chunk j; they are
            # interleaved into the following chunk (extra PE filler).
            xT = big.tile([P, NCH, S], BF16, tag="share1")  # reuses keyT slot

            def emit_sc_kt(m, j, kt, pt_pair):
                """scoresT k-tile kt for BOTH heads of pair m (q-chunk j).

                The two heads sit at partition offsets 0/64 -> their K=64
                matmuls land on row-tiles (0,0)/(64,0); emitted adjacently
                they stream through the PE array concurrently. One 2-bank
                psum holds both (bank per head); one exp evicts both."""
                sp = sc_ps.tile([P, 2, QC], F32, tag="sp", name=f"sp{m}{j}{kt}")
                for odd in range(2):
                    off = odd * DK
                    nc.tensor.matmul(
                        sp[:, odd, :],
                        kT[off : off + DK, m, kt * P : (kt + 1) * P],
                        qT[off : off + DK, m, j * QC : (j + 1) * QC],
                        start=True,
                        stop=True,
                    )
                nc.scalar.activation(pt_pair[:, kt, :, :], sp[:], EXP, scale=SCALE)

            xp_map = {}

            def emit_pv_mms(p, pt_pair, kcs):
                """p@v accumulation matmuls for position p over k-chunks kcs."""
                j, h = divmod(p, H)
                if p not in xp_map:
                    xp_map[p] = xt_ps.tile([DK + 1, QC], F32, tag="xp", name=f"xp{p}")
                xp = xp_map[p]
                for kc in kcs:
                    nc.tensor.matmul(
                        xp[:],
                        vpad[:, kc, h, :],
                        pt_pair[:, kc, h % 2, :],
                        start=(kc == 0),
                        stop=(kc == NCH - 1),
                    )

            def emit_pv_fin(p, dcols, xsb_map):
                """evict unnormalized x -> SBUF, denom row -> batch collector.

                Engine APs need 32-aligned start partitions, so the 4 denoms
                of a batch land at partitions 0/32/64/96 of one collector."""
                xp = xp_map.pop(p)
                xsb = xpool.tile([DK, QC], BF16, tag="xsb", name=f"xsb{p}")
                nc.vector.tensor_copy(xsb[:], xp[0:DK, :])
                b, r = divmod(p, 4)
                if r == 0:
                    dcols[b] = rpool.tile([97, QC], F32, tag="dcol", name=f"dc{b}")
                    nc.vector.memset(dcols[b][:], 1.0)  # only rows 0/32/64/96 matter
                nc.vector.tensor_copy(dcols[b][32 * r : 32 * r + 1, :], xp[DK : DK + 1, :])
                xsb_map[p] = xsb

            def emit_recip_half(j, pb, half, dcols, rcols):
                """half of the batch reciprocal (split so the DVE queue never
                blocks >~1.7us in front of the xp-slot-freeing copies)."""
                if half == 0:
                    rcols[pb // 4] = rpool.tile([97, QC], F32, tag="rcol", name=f"rc{pb}")
                sl = slice(half * (QC // 2), (half + 1) * (QC // 2))
                nc.vector.reciprocal(rcols[pb // 4][:, sl], dcols[pb // 4][:, sl])

            def emit_bcast(j, p, rcols, rb_map):
                r = 32 * (p % 4)
                # partition_broadcast ucode reads via Q7 core 0 -> input
                # must sit at base partition 0; bounce the row there.
                rb0 = rpool.tile([1, QC], F32, tag="rb0", name=f"rb0_{p}")
                nc.vector.tensor_copy(rb0[:], rcols[p // 4][r : r + 1, :])
                rb = rpool.tile([DK, QC], F32, tag="rb", name=f"rb{p}", bufs=6)
                nc.gpsimd.partition_broadcast(rb[:], rb0[:])
                rb_map[p] = rb

            def emit_norm_mult(j, p, xsb_map, rb_map):
                """deferred multiply (by now the broadcast is long done)."""
                ch, off = divmod(p % H, 2)
                off *= DK
                nc.vector.tensor_tensor(
                    xT[off : off + DK, ch, j * QC : (j + 1) * QC],
                    xsb_map[p][:],
                    rb_map[p][:],
                    op=MULT,
                )

            op_ps = {}

            def emit_oproj(t, half=None):
                """O = x @ Wo.T + bo for s-tile t (needs all of xT cols of t)."""
                if half in (0, None):
                    op_ps[t] = [
                        proj_ps.tile([P, QC], F32, tag="pp", name=f"op{t}_{j}")
                        for j in range(NQC)
                    ]
                ps = op_ps[t]
                cs = range(NCH) if half is None else range(half * 4, half * 4 + 4)
                for c in cs:
                    st = xT[:, c, t * P : (t + 1) * P]
                    for j in range(NQC):
                        nc.tensor.matmul(
                            ps[j][:],
                            st,
                            wo[:, c, j * QC : (j + 1) * QC],
                            start=(c == 0),
                            stop=(c == NCH - 1),
                        )
                if half == 0:
                    return
                for j in range(NQC):
                    ot = opool.tile([P, QC], F32, tag="ot", name=f"ot{t}_{j}")
                    nc.vector.tensor_tensor(
                        ot[:], ps[j][:], bo_b[:, j * QC : (j + 1) * QC], op=ADD
                    )
                    nc.sync.dma_start(
                        out=out_d[t * P : (t + 1) * P, j * QC : (j + 1) * QC],
                        in_=ot[:],
                    )

            # ---- flat 32-step pipeline over (chunk, head) ----
            # position p = 16*j + h. Norm chain of each 4-head batch is
            # spread one small op-group per later step (crossing chunk
            # boundaries) so no engine queue ever blocks the PE for long.
            dcols = {}
            xsb_map = {}
            rcols = {}
            rb_map = {}
            NPOS = NQC * H

            def norm_step(s):
                for p, acts in (
                    (s - 4, "r0"),
                    (s - 5, "r1"),
                    (s - 6, "b01"),
                    (s - 7, "b23m0"),
                    (s - 8, "m12"),
                    (s - 9, "m3"),
                ):
                    if p < 0 or p % 4 != 0 or p >= NPOS:
                        continue
                    j = p // H
                    if acts == "r0":
                        emit_recip_half(j, p, 0, dcols, rcols)
                    elif acts == "r1":
                        emit_recip_half(j, p, 1, dcols, rcols)
                    elif acts == "b01":
                        emit_bcast(j, p, rcols, rb_map)
                        emit_bcast(j, p + 1, rcols, rb_map)
                    elif acts == "b23m0":
                        emit_bcast(j, p + 2, rcols, rb_map)
                        emit_bcast(j, p + 3, rcols, rb_map)
                        emit_norm_mult(j, p, xsb_map, rb_map)
                    elif acts == "m12":
                        emit_norm_mult(j, p + 1, xsb_map, rb_map)
                        emit_norm_mult(j, p + 2, xsb_map, rb_map)
                    else:
                        emit_norm_mult(j, p + 3, xsb_map, rb_map)

            # pair-step loop: heads 2m/2m+1 processed together. kt score
            # matmuls (concurrent row-tiles) are woven between the previous
            # pair's p@v matmuls so the in-order PE queue never parks behind
            # an exp wait; the filler at step start covers the ACT backlog.
            pt_map = {}
            KC_A = tuple(range(NCH // 2))
            KC_B = tuple(range(NCH // 2, NCH))
            for ps_ in range(NPOS // 2):
                j, m = divmod(ps_, H // 2)
                fillers = []
                if ps_ == 0:  # V tiles 4,5 hide under this step's exp window
                    fillers += [(emit_vproj, 4, 0), (emit_vproj, 4, 1),
                                (emit_vproj, 5, 0), (emit_vproj, 5, 1)]
                elif ps_ == 1:  # V tiles 6,7 — evictions land before the
                    # KC_B p@v of this step reads their k-chunks
                    fillers += [(emit_vproj, 6, 0), (emit_vproj, 6, 1),
                                (emit_vproj, 7, 0), (emit_vproj, 7, 1)]
                if j == 0 and m < 7:
                    fillers += [(emit_ktproj, m + 1, 0), (emit_ktproj, m + 1, 1)]
                elif j == 1 and m in (3, 4, 5):
                    fillers += [(emit_oproj, m - 3, 0), (emit_oproj, m - 3, 1)]

                def filler(i):
                    if i < len(fillers):
                        f, a, b = fillers[i]
                        f(a, b)

                pt_pair = big.tile(
                    [P, NCH, 2, QC], BF16, tag="ptv", bufs=2, name=f"ptp{ps_}"
                )
                d0, d1 = 2 * ps_ - 2, 2 * ps_ - 1
                pp_prev = pt_map.pop(ps_ - 1, None)
                filler(0)
                emit_sc_kt(m, j, 0, pt_pair)
                filler(1)
                emit_sc_kt(m, j, 1, pt_pair)
                filler(2)
                emit_sc_kt(m, j, 2, pt_pair)
                filler(3)
                if pp_prev is not None:
                    emit_pv_mms(d0, pp_prev, KC_A)
                emit_sc_kt(m, j, 3, pt_pair)
                if pp_prev is not None:
                    emit_pv_mms(d0, pp_prev, KC_B)
                    emit_pv_fin(d0, dcols, xsb_map)
                filler(4)
                emit_sc_kt(m, j, 4, pt_pair)
                if pp_prev is not None:
                    emit_pv_mms(d1, pp_prev, KC_A)
                emit_sc_kt(m, j, 5, pt_pair)
                if pp_prev is not None:
                    emit_pv_mms(d1, pp_prev, KC_B)
                    emit_pv_fin(d1, dcols, xsb_map)
                filler(5)
                emit_sc_kt(m, j, 6, pt_pair)
                emit_sc_kt(m, j, 7, pt_pair)
                for i in range(6, len(fillers)):
                    filler(i)
                pt_map[ps_] = pt_pair
                norm_step(2 * ps_)
                norm_step(2 * ps_ + 1)
            pp_last = pt_map.pop(NPOS // 2 - 1)
            emit_pv_mms(NPOS - 2, pp_last, KC_A + KC_B)
            emit_pv_fin(NPOS - 2, dcols, xsb_map)
            emit_pv_mms(NPOS - 1, pp_last, KC_A + KC_B)
            emit_pv_fin(NPOS - 1, dcols, xsb_map)
            # chunk-0-dependent tile held back: PE work covering last chain
            emit_oproj(3)
            for s in range(NPOS, NPOS + 8):
                norm_step(s)
            # tail: O-proj s-tiles of the last q-chunk
            for t in range(4, NCH):
                emit_oproj(t)

    nc.finalize()
    return nc


def get_nc():
    if "nc" not in _CACHE:
        _CACHE["nc"] = _build_nc()
    return _CACHE["nc"]


def _tp_bf16(a):
    """[X, Y] f32 -> transposed bf16 [NCH, P, Y]."""
    return (
        np.ascontiguousarray(np.asarray(a, dtype=np.float32).T)
        .astype(ml_dtypes.bfloat16)
        .reshape(NCH, P, -1)
    )


def make_in_maps(query, key, value, Wk, bk, Wv, bv, Wo, bo):
    wkT = _tp_bf16(Wk)
    wvT = _tp_bf16(Wv)
    woT = _tp_bf16(Wo)
    bk = np.asarray(bk, dtype=np.float32)
    bv = np.asarray(bv, dtype=np.float32)
    bo = np.asarray(bo, dtype=np.float32)
    in_maps = []
    for b in range(B):
        in_maps.append(
            {
                "qT": _tp_bf16(query[b]),
                "keyT": _tp_bf16(key[b]),
                "valT": _tp_bf16(value[b]),
                "wkT": wkT,
                "wvT": wvT,
                "woT": woT,
                "bk": bk,
                "bv": bv,
                "bo": bo,
            }
        )
    return in_maps


def run(trace=False, **inputs):
    from concourse.bass_utils import run_bass_kernel_spmd

    nc = get_nc()
    in_maps = make_in_maps(**inputs)
    res = run_bass_kernel_spmd(nc, in_maps, list(range(N_CORES)), trace=trace)
    out = np.stack([res.results[i]["out"] for i in range(N_CORES)], axis=0)
    return out, res


def kernel(**inputs):
    out, _ = run(trace=False, **inputs)
    return out


# revision 61
# speedup vs baseline: 1.0215x; 1.0215x over previous
"""Multi-headed attention (B=8, S=1024, D=1024, H=16) on 8 TRN2 NeuronCores.

Strategy: pure data parallelism over the batch — core b computes batch element b
end-to-end (no collectives). All matmuls in bf16 (fp32 PSUM accumulation).

Per-core dataflow (everything "T" is feature-major [D, S]):
  inputs (host-pretransposed, bf16): qT, keyT, valT, wkT, wvT, woT
  1. K_T[d_out, s]  = matmul(lhsT=wkT, rhs=keyT) + bk      (bias per-partition)
  2. V[s, d_out]    = matmul(lhsT=valT, rhs=wvT) + bv      -> packed [s, h, 65]
                      with a ones column per head (gives softmax denominators
                      for free inside the p@v matmul)
  3. per head h: scoresT[k, q] = matmul(lhsT=K_T_h[64,128], rhs=qT_h[64,512])
                 pT = exp(scoresT / 8)  (ACT; max-subtraction skipped — scores
                 are provably small for this problem)
  4. xT_h[65, q] accum = matmul(lhsT=[V_h | 1][128,65], rhs=pT[128,512]);
     row 64 = softmax denominator; normalize rows 0..63 by its reciprocal
     (partition-broadcast via DMA)
  5. O[s, d_out] = matmul(lhsT=xT, rhs=woT) + bo -> DMA out (f32)
"""

import numpy as np
import ml_dtypes

import concourse.bass as bass
import concourse.bacc as bacc
import concourse.mybir as mybir
import concourse.tile as tile
from contextlib import ExitStack

B, S, D, H = 8, 1024, 1024, 16
P = 128
DK = D // H          # 64
NCH = D // P         # 8
QC = 512             # free-dim chunk (one PSUM bank)
NQC = S // QC        # 2
SCALE = 1.0 / float(np.sqrt(DK))  # 0.125
N_CORES = 8

BF16 = mybir.dt.bfloat16
F32 = mybir.dt.float32
ADD = mybir.AluOpType.add
MULT = mybir.AluOpType.mult
EXP = mybir.ActivationFunctionType.Exp

_CACHE = {}


def _build_nc():
    nc = bacc.Bacc(None)

    qT_d = nc.dram_tensor("qT", [NCH, P, S], BF16, kind="ExternalInput")
    keyT_d = nc.dram_tensor("keyT", [NCH, P, S], BF16, kind="ExternalInput")
    valT_d = nc.dram_tensor("valT", [NCH, P, S], BF16, kind="ExternalInput")
    wkT_d = nc.dram_tensor("wkT", [NCH, P, D], BF16, kind="ExternalInput")
    wvT_d = nc.dram_tensor("wvT", [NCH, P, D], BF16, kind="ExternalInput")
    woT_d = nc.dram_tensor("woT", [NCH, P, D], BF16, kind="ExternalInput")
    bk_d = nc.dram_tensor("bk", [D], F32, kind="ExternalInput")
    bv_d = nc.dram_tensor("bv", [D], F32, kind="ExternalInput")
    bo_d = nc.dram_tensor("bo", [D], F32, kind="ExternalInput")
    out_d = nc.dram_tensor("out", [S, D], F32, kind="ExternalOutput")

    with tile.TileContext(nc) as tc:
        with ExitStack() as ctx:
            const = ctx.enter_context(tc.tile_pool(name="const", bufs=1))
            big = ctx.enter_context(tc.tile_pool(name="big", bufs=1))
            wpool = ctx.enter_context(tc.tile_pool(name="wpool", bufs=1))
            ppool = ctx.enter_context(tc.tile_pool(name="ppool", bufs=2))
            opool = ctx.enter_context(tc.tile_pool(name="opool", bufs=4))
            rpool = ctx.enter_context(tc.tile_pool(name="rpool", bufs=2))
            xpool = ctx.enter_context(tc.tile_pool(name="xpool", bufs=10))
            proj_ps = ctx.enter_context(
                tc.tile_pool(name="proj_ps", bufs=2, space="PSUM")
            )
            sc_ps = ctx.enter_context(tc.tile_pool(name="sc_ps", bufs=2, space="PSUM"))
            xt_ps = ctx.enter_context(tc.tile_pool(name="xt_ps", bufs=2, space="PSUM"))

            # --- SBUF resident tensors ---
            qT = big.tile([P, NCH, S], BF16, tag="qT")
            keyT = big.tile([P, NCH, S], BF16, tag="share1")  # reused later by xT
            # valT in two s-halves; the same 2-slot/16KB tag later hosts the
            # paired-head pT tiles (valT is dead after the V projection)
            valT_a = big.tile([P, NCH, QC], BF16, tag="ptv", bufs=2)
            valT_b = big.tile([P, NCH, QC], BF16, tag="ptv", bufs=2)
            kT = big.tile([P, NCH, S], BF16, tag="kT")
            vpad = big.tile([P, NCH, H, DK + 1], BF16, tag="vpad")
            wk = wpool.tile([P, NCH, D], BF16, tag="wk")
            wv = wpool.tile([P, NCH, D], BF16, tag="wv")
            wo = wpool.tile([P, NCH, D], BF16, tag="wo")
            bk_sb = const.tile([P, NCH], F32, tag="bk")
            bv_b = const.tile([P, D], F32, tag="bv")
            bo_b = const.tile([P, D], F32, tag="bo")

            # --- input DMAs (V-proj deps first: it must fully precede p@v) ---
            nc.sync.dma_start(
                out=valT_a[:], in_=valT_d[:, :, 0:QC].rearrange("c p f -> p c f")
            )
            nc.sync.dma_start(
                out=wv[:, :, 0:QC], in_=wvT_d[:, :, 0:QC].rearrange("c p f -> p c f")
            )
            nc.sync.dma_start(
                out=valT_b[:], in_=valT_d[:, :, QC:S].rearrange("c p f -> p c f")
            )
            nc.sync.dma_start(
                out=wv[:, :, QC:D], in_=wvT_d[:, :, QC:D].rearrange("c p f -> p c f")
            )
            nc.sync.dma_start(out=bv_b[:], in_=bv_d[:][None, :].to_broadcast((P, D)))
            nc.sync.dma_start(out=keyT[:], in_=keyT_d[:].rearrange("c p f -> p c f"))
            nc.sync.dma_start(out=wk[:], in_=wkT_d[:].rearrange("c p f -> p c f"))
            nc.sync.dma_start(out=bk_sb[:], in_=bk_d[:].rearrange("(c p) -> p c", p=P))
            nc.sync.dma_start(out=qT[:], in_=qT_d[:].rearrange("c p f -> p c f"))
            nc.sync.dma_start(out=wo[:], in_=woT_d[:].rearrange("c p f -> p c f"))
            nc.sync.dma_start(out=bo_b[:], in_=bo_d[:][None, :].to_broadcast((P, D)))

            # --- 1. K_T = Wk @ key.T + bk  (feature-major) ---
            # m-tile 0 runs in the prologue; tiles 1..7 are interleaved into
            # attention chunk 0 as PE filler (head pair m needs only tile m).
            ktp_ps = {}

            def emit_ktproj(m, half=None):
                if half in (0, None):
                    ktp_ps[m] = [
                        proj_ps.tile([P, QC], F32, tag="pp", name=f"kp{m}_{j}")
                        for j in range(NQC)
                    ]
                ps = ktp_ps[m]
                cs = range(NCH) if half is None else range(half * 4, half * 4 + 4)
                for c in cs:  # d_in chunk (contraction)
                    st = wk[:, c, m * P : (m + 1) * P]
                    for j in range(NQC):
                        nc.tensor.matmul(
                            ps[j][:],
                            st,
                            keyT[:, c, j * QC : (j + 1) * QC],
                            start=(c == 0),
                            stop=(c == NCH - 1),
                        )
                if half in (1, None):
                    for j in range(NQC):
                        nc.vector.tensor_scalar_add(
                            kT[:, m, j * QC : (j + 1) * QC],
                            ps[j][:],
                            bk_sb[:, m : m + 1],
                        )

            # --- 2. V = value @ Wv.T + bv (token-major, head-padded w/ ones) ---
            # tiles t0-3 run in the prologue; t4-7 hide inside attention
            # steps 0-1 as PE filler (p@v starts at step 1 and consumes
            # k-chunks 4-7 only after those tiles' evictions are emitted)
            vp_ps = {}

            def emit_vproj(t, half):
                if half == 0:
                    vp_ps[t] = [
                        proj_ps.tile([P, QC], F32, tag="pp", name=f"vp{t}_{j}")
                        for j in range(NQC)
                    ]
                ps = vp_ps[t]
                vhalf = valT_a if t < 4 else valT_b
                j = half
                for c in range(NCH):
                    nc.tensor.matmul(
                        ps[j][:],
                        vhalf[:, c, (t % 4) * P : (t % 4 + 1) * P],
                        wv[:, c, j * QC : (j + 1) * QC],
                        start=(c == 0),
                        stop=(c == NCH - 1),
                    )
                if half == 0:
                    return
                hpc = QC // DK  # heads per psum chunk
                for j in range(NQC):
                    nc.vector.tensor_tensor(
                        vpad[:, t, j * hpc : (j + 1) * hpc, 0:DK],
                        ps[j][:].rearrange("p (h d) -> p h d", d=DK),
                        bv_b[:, j * QC : (j + 1) * QC].rearrange(
                            "p (h d) -> p h d", d=DK
                        ),
                        op=ADD,
                    )
                nc.vector.memset(vpad[:, t, :, DK : DK + 1], 1.0)

            for t in range(4):
                emit_vproj(t, 0)
                emit_vproj(t, 1)

            emit_ktproj(0)  # head pair 0's K_T; tiles 1..7 interleave below

            # --- 3+4. per-head attention, q-chunk-outer, head-pipelined ---
            # PE is in-order: emit scores(h) before p@v(h-1) so the PE has
            # work while ACT chews through exp(h-1). O-projection s-tiles
            # for q-chunk j unlock once all heads finished chunk j; they are
            # interleaved into the following chunk (extra PE filler).
            xT = big.tile([P, NCH, S], BF16, tag="share1")  # reuses keyT slot

            def emit_sc_kt(m, j, kt, pt_pair):
                """scoresT k-tile kt for BOTH heads of pair m (q-chunk j).

                The two heads sit at partition offsets 0/64 -> their K=64
                matmuls land on row-tiles (0,0)/(64,0); emitted adjacently
                they stream through the PE array concurrently. One 2-bank
                psum holds both (bank per head); one exp evicts both."""
                sp = sc_ps.tile([P, 2, QC], F32, tag="sp", name=f"sp{m}{j}{kt}")
                for odd in range(2):
                    off = odd * DK
                    nc.tensor.matmul(
                        sp[:, odd, :],
                        kT[off : off + DK, m, kt * P : (kt + 1) * P],
                        qT[off : off + DK, m, j * QC : (j + 1) * QC],
                        start=True,
                        stop=True,
                    )
                nc.scalar.activation(pt_pair[:, kt, :, :], sp[:], EXP, scale=SCALE)

            xp_map = {}

            def emit_pv_mms(p, pt_pair, kcs):
                """p@v accumulation matmuls for position p over k-chunks kcs."""
                j, h = divmod(p, H)
                if p not in xp_map:
                    xp_map[p] = xt_ps.tile([DK + 1, QC], F32, tag="xp", name=f"xp{p}")
                xp = xp_map[p]
                for kc in kcs:
                    nc.tensor.matmul(
                        xp[:],
                        vpad[:, kc, h, :],
                        pt_pair[:, kc, h % 2, :],
                        start=(kc == 0),
                        stop=(kc == NCH - 1),
                    )

            def emit_pv_fin(p, dcols, xsb_map):
                """evict unnormalized x -> SBUF, denom row -> batch collector.

                Engine APs need 32-aligned start partitions, so the 4 denoms
                of a batch land at partitions 0/32/64/96 of one collector."""
                xp = xp_map.pop(p)
                xsb = xpool.tile([DK, QC], BF16, tag="xsb", name=f"xsb{p}")
                nc.vector.tensor_copy(xsb[:], xp[0:DK, :])
                b, r = divmod(p, 4)
                if p >= 30:  # last two heads: own 2-head batch (shorter tail chain)
                    b, r = 8, p - 30
                if r == 0:
                    dcols[b] = rpool.tile([97, QC], F32, tag="dcol", name=f"dc{b}")
                    nc.vector.memset(dcols[b][:], 1.0)  # only rows 0/32/64/96 matter
                nc.vector.tensor_copy(dcols[b][32 * r : 32 * r + 1, :], xp[DK : DK + 1, :])
                xsb_map[p] = xsb

            def emit_recip_half(j, pb, half, dcols, rcols):
                """half of the batch reciprocal (split so the DVE queue never
                blocks >~1.7us in front of the xp-slot-freeing copies)."""
                if half == 0:
                    rcols[pb // 4] = rpool.tile([97, QC], F32, tag="rcol", name=f"rc{pb}")
                sl = slice(half * (QC // 2), (half + 1) * (QC // 2))
                nc.vector.reciprocal(rcols[pb // 4][:, sl], dcols[pb // 4][:, sl])

            def emit_bcast(j, p, rcols, rb_map):
                r = 32 * (p % 4)
                # partition_broadcast ucode reads via Q7 core 0 -> input
                # must sit at base partition 0; bounce the row there.
                rb0 = rpool.tile([1, QC], F32, tag="rb0", name=f"rb0_{p}")
                nc.vector.tensor_copy(rb0[:], rcols[p // 4][r : r + 1, :])
                rb = rpool.tile([DK, QC], F32, tag="rb", name=f"rb{p}", bufs=6)
                nc.gpsimd.partition_broadcast(rb[:], rb0[:])
                rb_map[p] = rb

            def emit_norm_mult(j, p, xsb_map, rb_map):
                """deferred multiply (by now the broadcast is long done)."""
                ch, off = divmod(p % H, 2)
                off *= DK
                nc.vector.tensor_tensor(
                    xT[off : off + DK, ch, j * QC : (j + 1) * QC],
                    xsb_map[p][:],
                    rb_map[p][:],
                    op=MULT,
                )

            op_ps = {}

            def emit_oproj(t, half=None):
                """O = x @ Wo.T + bo for s-tile t (needs all of xT cols of t)."""
                if half in (0, None):
                    op_ps[t] = [
                        proj_ps.tile([P, QC], F32, tag="pp", name=f"op{t}_{j}")
                        for j in range(NQC)
                    ]
                ps = op_ps[t]
                cs = range(NCH) if half is None else range(half * 4, half * 4 + 4)
                for c in cs:
                    st = xT[:, c, t * P : (t + 1) * P]
                    for j in range(NQC):
                        nc.tensor.matmul(
                            ps[j][:],
                            st,
                            wo[:, c, j * QC : (j + 1) * QC],
                            start=(c == 0),
                            stop=(c == NCH - 1),
                        )
                if half == 0:
                    return
                for j in range(NQC):
                    ot = opool.tile([P, QC], F32, tag="ot", name=f"ot{t}_{j}")
                    nc.vector.tensor_tensor(
                        ot[:], ps[j][:], bo_b[:, j * QC : (j + 1) * QC], op=ADD
                    )
                    nc.sync.dma_start(
                        out=out_d[t * P : (t + 1) * P, j * QC : (j + 1) * QC],
                        in_=ot[:],
                    )

            # ---- flat 32-step pipeline over (chunk, head) ----
            # position p = 16*j + h. Norm chain of each 4-head batch is
            # spread one small op-group per later step (crossing chunk
            # boundaries) so no engine queue ever blocks the PE for long.
            dcols = {}
            xsb_map = {}
            rcols = {}
            rb_map = {}
            NPOS = NQC * H

            def norm_step(s):
                for p, acts in (
                    (s - 4, "r0"),
                    (s - 5, "r1"),
                    (s - 6, "b01"),
                    (s - 7, "b23m0"),
                    (s - 8, "m12"),
                    (s - 9, "m3"),
                ):
                    if p < 0 or p % 4 != 0 or p >= NPOS or p == 28:
                        continue
                    j = p // H
                    if acts == "r0":
                        emit_recip_half(j, p, 0, dcols, rcols)
                    elif acts == "r1":
                        emit_recip_half(j, p, 1, dcols, rcols)
                    elif acts == "b01":
                        emit_bcast(j, p, rcols, rb_map)
                        emit_bcast(j, p + 1, rcols, rb_map)
                    elif acts == "b23m0":
                        emit_bcast(j, p + 2, rcols, rb_map)
                        emit_bcast(j, p + 3, rcols, rb_map)
                        emit_norm_mult(j, p, xsb_map, rb_map)
                    elif acts == "m12":
                        emit_norm_mult(j, p + 1, xsb_map, rb_map)
                        emit_norm_mult(j, p + 2, xsb_map, rb_map)
                    else:
                        emit_norm_mult(j, p + 3, xsb_map, rb_map)

            # pair-step loop: heads 2m/2m+1 processed together. kt score
            # matmuls (concurrent row-tiles) are woven between the previous
            # pair's p@v matmuls so the in-order PE queue never parks behind
            # an exp wait; the filler at step start covers the ACT backlog.
            pt_map = {}
            KC_A = tuple(range(NCH // 2))
            KC_B = tuple(range(NCH // 2, NCH))
            for ps_ in range(NPOS // 2):
                j, m = divmod(ps_, H // 2)
                fillers = []
                if ps_ == 0:  # V tiles 4,5 hide under this step's exp window
                    fillers += [(emit_vproj, 4, 0), (emit_vproj, 4, 1),
                                (emit_vproj, 5, 0), (emit_vproj, 5, 1)]
                elif ps_ == 1:  # V tiles 6,7 — evictions land before the
                    # KC_B p@v of this step reads their k-chunks
                    fillers += [(emit_vproj, 6, 0), (emit_vproj, 6, 1),
                                (emit_vproj, 7, 0), (emit_vproj, 7, 1)]
                if j == 0 and m < 7:
                    fillers += [(emit_ktproj, m + 1, 0), (emit_ktproj, m + 1, 1)]
                elif j == 1 and m in (3, 4, 5):
                    fillers += [(emit_oproj, m - 3, 0), (emit_oproj, m - 3, 1)]

                def filler(i):
                    if i < len(fillers):
                        f, a, b = fillers[i]
                        f(a, b)

                pt_pair = big.tile(
                    [P, NCH, 2, QC], BF16, tag="ptv", bufs=2, name=f"ptp{ps_}"
                )
                d0, d1 = 2 * ps_ - 2, 2 * ps_ - 1
                pp_prev = pt_map.pop(ps_ - 1, None)
                filler(0)
                emit_sc_kt(m, j, 0, pt_pair)
                filler(1)
                emit_sc_kt(m, j, 1, pt_pair)
                filler(2)
                emit_sc_kt(m, j, 2, pt_pair)
                filler(3)
                if pp_prev is not None:
                    emit_pv_mms(d0, pp_prev, KC_A)
                emit_sc_kt(m, j, 3, pt_pair)
                if pp_prev is not None:
                    emit_pv_mms(d0, pp_prev, KC_B)
                    emit_pv_fin(d0, dcols, xsb_map)
                filler(4)
                emit_sc_kt(m, j, 4, pt_pair)
                if pp_prev is not None:
                    emit_pv_mms(d1, pp_prev, KC_A)
                emit_sc_kt(m, j, 5, pt_pair)
                if pp_prev is not None:
                    emit_pv_mms(d1, pp_prev, KC_B)
                    emit_pv_fin(d1, dcols, xsb_map)
                filler(5)
                emit_sc_kt(m, j, 6, pt_pair)
                emit_sc_kt(m, j, 7, pt_pair)
                for i in range(6, len(fillers)):
                    filler(i)
                pt_map[ps_] = pt_pair
                norm_step(2 * ps_)
                norm_step(2 * ps_ + 1)
            # tail: batch 7 (pos 28-29, fins landed inside step 15) chains
            # interleave with the final p@v pair; pos 30-31 form batch 8
            pp_last = pt_map.pop(NPOS // 2 - 1)
            emit_recip_half(1, 28, 0, dcols, rcols)
            emit_pv_mms(NPOS - 2, pp_last, KC_A + KC_B)
            emit_recip_half(1, 28, 1, dcols, rcols)
            emit_pv_fin(NPOS - 2, dcols, xsb_map)
            emit_bcast(1, 28, rcols, rb_map)
            emit_bcast(1, 29, rcols, rb_map)
            emit_pv_mms(NPOS - 1, pp_last, KC_A + KC_B)
            emit_norm_mult(1, 28, xsb_map, rb_map)
            emit_norm_mult(1, 29, xsb_map, rb_map)
            emit_pv_fin(NPOS - 1, dcols, xsb_map)
            # chunk-0-dependent tile held back: PE work covering last chain
            emit_oproj(3)
            # batch 8 chain (rows 0/32 of dcols[8])
            rcol8 = rpool.tile([33, QC], F32, tag="rcol", name="rc8")
            nc.vector.reciprocal(rcol8[:, 0 : QC // 2], dcols[8][0:33, 0 : QC // 2])
            nc.vector.reciprocal(rcol8[:, QC // 2 : QC], dcols[8][0:33, QC // 2 : QC])
            for i, pf in enumerate((30, 31)):
                rb0f = rpool.tile([1, QC], F32, tag="rb0", name=f"rb0f{pf}")
                nc.vector.tensor_copy(rb0f[:], rcol8[32 * i : 32 * i + 1, :])
                rbf = rpool.tile([DK, QC], F32, tag="rb", name=f"rbf{pf}", bufs=6)
                nc.gpsimd.partition_broadcast(rbf[:], rb0f[:])
                ch, off = divmod(pf % H, 2)
                off *= DK
                nc.vector.tensor_tensor(
                    xT[off : off + DK, ch, QC : 2 * QC],
                    xsb_map[pf][:],
                    rbf[:],
                    op=MULT,
                )
            for s in range(NPOS, NPOS + 8):
                norm_step(s)
            # tail: O-proj s-tiles of the last q-chunk
            for t in range(4, NCH):
                emit_oproj(t)

    nc.finalize()
    return nc


def get_nc():
    if "nc" not in _CACHE:
        _CACHE["nc"] = _build_nc()
    return _CACHE["nc"]


def _tp_bf16(a):
    """[X, Y] f32 -> transposed bf16 [NCH, P, Y]."""
    return (
        np.ascontiguousarray(np.asarray(a, dtype=np.float32).T)
        .astype(ml_dtypes.bfloat16)
        .reshape(NCH, P, -1)
    )


def make_in_maps(query, key, value, Wk, bk, Wv, bv, Wo, bo):
    wkT = _tp_bf16(Wk)
    wvT = _tp_bf16(Wv)
    woT = _tp_bf16(Wo)
    bk = np.asarray(bk, dtype=np.float32)
    bv = np.asarray(bv, dtype=np.float32)
    bo = np.asarray(bo, dtype=np.float32)
    in_maps = []
    for b in range(B):
        in_maps.append(
            {
                "qT": _tp_bf16(query[b]),
                "keyT": _tp_bf16(key[b]),
                "valT": _tp_bf16(value[b]),
                "wkT": wkT,
                "wvT": wvT,
                "woT": woT,
                "bk": bk,
                "bv": bv,
                "bo": bo,
            }
        )
    return in_maps


def run(trace=False, **inputs):
    from concourse.bass_utils import run_bass_kernel_spmd

    nc = get_nc()
    in_maps = make_in_maps(**inputs)
    res = run_bass_kernel_spmd(nc, in_maps, list(range(N_CORES)), trace=trace)
    out = np.stack([res.results[i]["out"] for i in range(N_CORES)], axis=0)
    return out, res


def kernel(**inputs):
    out, _ = run(trace=False, **inputs)
    return out
